# revision 23
# baseline (speedup 1.0000x reference)
"""CenterCut2 Trainium2 kernel (v2).

For each sample b: find argmax of power = sum_c x[b,c]^2 over the (D,H,W)
volume, then extract the 16x32x32 window centered on the peak with circular
wraparound (equivalent to reference's per-sample roll + center crop).

Sharding: pure data parallelism, 4 samples per core across 8 cores.

Per-core device program (samples s=0..3, volumes v=2s+c laid out [128, 8192]
with flat voxel index = p*8192 + f = dd*16384 + hh*128 + w):

  1. Stream both channels in [128, 2, 2048] chunks; square in place on ACT;
     a single fused DVE tensor_tensor_reduce per 1024-wide sub-chunk both
     adds the channel squares into the power map AND emits the sub-chunk max
     (mb[128, 8]) -- no separate MAX8 pass.
  2. Global argmax via small reductions only: partition_all_reduce(max) for
     the global max M, an is_equal + iota + BIG-constant trick over mb to
     pick the lexicographically-lowest (partition, sub-chunk) containing M,
     then max_index over just the winning 1024-wide window (register-offset
     AP). Tie-breaks match jnp.argmax (lowest flat index).
  3. One 64-descriptor dma_gather (16KB each) pulls the two 32-row h-chunks
     per (channel, d-slice); rows land at partition chunk*32 + c*16 + d.
  4. Window extraction via 4 register-offset [32,32,32] copies into a padded
     [32, 64, 64] output tile: A/B split the h-window across the two 32-row
     chunks, C/D patch the w-wraparound columns (writes beyond the window
     land in tile padding). One strided DMA writes the sample's output.
"""
import os
import sys

sys.path.insert(0, "/opt/trn_rl_repo")

import numpy as np

_DBG = set(filter(None, os.environ.get("K_DBG", "").split(",")))

import concourse.bass as bass
import concourse.bacc as bacc
import concourse.mybir as mybir
from concourse.tile import TileContext
from concourse.tile_rust import add_dep_helper
from concourse.bass_utils import run_bass_kernel_spmd
from concourse.bass_isa import ReduceOp

F32 = mybir.dt.float32
I32 = mybir.dt.int32
I16 = mybir.dt.int16
U32 = mybir.dt.uint32
A = mybir.AluOpType
DVE = mybir.EngineType.DVE
ACT = mybir.EngineType.Activation

N_CORES = 8
S_PER_CORE = 4          # samples per core
N_VOLS = 2 * S_PER_CORE # channel volumes per core
VOL = 64 * 128 * 128    # voxels per volume
FREE = VOL // 128       # 8192 free elements per partition
CHUNK = 2048            # streaming chunk per channel (1 MiB per DMA)
SUB = 1024              # ttr accum granularity (8 sub-chunks per sample)
NSUB = FREE // SUB
BIG2 = float(1 << 22)   # > max flat index (2^20)
BIG3 = float(1 << 21)   # > max q index (1031)

_cache = {}


def _build():
    nc = bacc.Bacc("TRN2", target_bir_lowering=False, debug=False, num_devices=N_CORES)
    x = nc.dram_tensor("x", [N_VOLS, 128, FREE], F32, kind="ExternalInput")
    y = nc.dram_tensor("y", [128, 1024], F32, kind="ExternalOutput")

    iota_base_c = nc.inline_tensor(
        (np.arange(128, dtype=np.float32) * FREE).reshape(128, 1), name="iota_base"
    )
    iota_q8_c = nc.inline_tensor(
        (np.arange(128, dtype=np.float32)[:, None] * NSUB
         + np.arange(NSUB, dtype=np.float32)[None, :]),
        name="iota_q8",
    )
    iotaq_c = nc.inline_tensor(np.arange(16, dtype=np.int32).reshape(16, 1), name="iotaq")
    # gather source view: [2048 rows, 4096] -- row = vol*256 + dd*4 + hchunk
    xrows = x.ap().rearrange("v p (a b) -> (v p a) b", a=2)

    with TileContext(nc) as tc:
        with (
            tc.tile_pool(name="xc", bufs=3) as xpool,
            tc.tile_pool(name="pw", bufs=2) as ppool,
            tc.tile_pool(name="sm", bufs=2) as spool,
            tc.tile_pool(name="ob", bufs=2) as opool,
            tc.tile_pool(name="big", bufs=1) as bpool,
        ):
            base = bpool.tile([128, 1], F32, tag="base")
            nc.sync.dma_start(base[:, :], iota_base_c.ap()[:, :])
            iq8 = bpool.tile([128, NSUB], F32, tag="iq8")
            nc.sync.dma_start(iq8[:, :], iota_q8_c.ap()[:, :])
            iotaq = bpool.tile([16, 1], I32, tag="iotaq")
            nc.sync.dma_start(iotaq[:, :], iotaq_c.ap()[:, :])
            scal = bpool.tile([1, 64], I32, tag="scal")
            # persistent gather landing tile; zero once so the dynamic
            # over-reads in the extraction copies never touch uninit memory
            # (padded past 64*128 so the overlapping 160-wide view fits)
            G = bpool.tile([128, FREE + 256], F32, tag="gt")
            nc.vector.memset(G[:, :], 0.0)

            def ts(dst, src, s1, op0, s2=None, op1=None):
                kw = {}
                if s2 is not None:
                    kw = dict(scalar2=s2, op1=op1)
                else:
                    kw = dict(scalar2=None)
                return nc.vector.tensor_scalar(
                    out=dst, in0=src, scalar1=s1, op0=op0, **kw
                )

            for s in range(S_PER_CORE):
                power = ppool.tile([128, FREE], F32, tag="pw")
                m8 = spool.tile([128, NSUB * 8], F32, tag="m8")
                # [128, NSUB] view of the per-sub-chunk maxes (stride 8: first
                # element of each vector.max output group)
                _mbb = m8[:, 0:NSUB]
                mb = bass.AP(_mbb.tensor, _mbb.offset, [list(_mbb.ap[0])] + [[8, NSUB]])
                # stream both channels, square in place, fused add+max
                for k in range(FREE // CHUNK):
                    sl = slice(k * CHUNK, (k + 1) * CHUNK)
                    xc = xpool.tile([128, 2, CHUNK], F32, tag="xc")
                    nc.sync.dma_start(xc[:, 0, :], x[2 * s, :, sl])
                    nc.sync.dma_start(xc[:, 1, :], x[2 * s + 1, :, sl])
                    xflat = xc[:, :, :].rearrange("p c f -> p (c f)")
                    nc.scalar.square(xflat, xflat)  # in place
                    for j in range(CHUNK // SUB):
                        c = k * (CHUNK // SUB) + j
                        jj = slice(j * SUB, (j + 1) * SUB)
                        if "ttr" in _DBG:
                            nc.vector.tensor_tensor_reduce(
                                out=power[:, c * SUB : (c + 1) * SUB],
                                in0=xc[:, 0, jj],
                                in1=xc[:, 1, jj],
                                scale=1.0,
                                scalar=-1.0,
                                op0=A.add,
                                op1=A.max,
                                accum_out=m8[:, 8 * c : 8 * c + 1],
                            )
                        else:
                            nc.vector.tensor_tensor(
                                out=power[:, c * SUB : (c + 1) * SUB],
                                in0=xc[:, 0, jj], in1=xc[:, 1, jj], op=A.add,
                            )
                            nc.vector.max(
                                out=m8[:, 8 * c : 8 * c + 8],
                                in_=power[:, c * SUB : (c + 1) * SUB],
                            )

                # ---- global argmax via small reductions ----
                pm8 = spool.tile([128, 8], F32, tag="pm8")
                nc.vector.max(out=pm8[:, :], in_=mb[:, :])
                m = pm8[:, 0:1]
                M = spool.tile([128, 1], F32, tag="M")
                nc.gpsimd.partition_all_reduce(M[:, :], m, 128, ReduceOp.max)
                eq = spool.tile([128, 1], F32, tag="eq")
                nc.vector.tensor_tensor(out=eq[:, :], in0=m, in1=M[:, :], op=A.is_equal)

                # winning (partition, sub-chunk), lexicographic min
                eqc = spool.tile([128, NSUB], F32, tag="eqc")
                mb_b, M_b = bass.broadcast_tensor_aps(mb[:, :], M[:, :])
                nc.vector.tensor_tensor(out=eqc[:, :], in0=mb_b, in1=M_b, op=A.is_equal)
                candq = spool.tile([128, NSUB], F32, tag="candq")
                nc.vector.scalar_tensor_tensor(
                    out=candq[:, :], in0=eqc[:, :], scalar=BIG3, in1=iq8[:, :],
                    op0=A.mult, op1=A.subtract,
                )
                cq8 = spool.tile([128, 8], F32, tag="cq8")
                nc.vector.max(out=cq8[:, :], in_=candq[:, :])
                allq = spool.tile([128, 1], F32, tag="allq")
                nc.gpsimd.partition_all_reduce(allq[:, :], cq8[:, 0:1], 128, ReduceOp.max)

                def C(j):
                    return scal[:, 16 * s + j : 16 * s + j + 1]

                # q = BIG3 - allq; c1024 = (q & 7) << 10
                qf = spool.tile([1, 1], F32, tag="qf")
                ts(qf[:, :], allq[0:1, 0:1], BIG3, A.subtract, -1.0, A.mult)
                nc.vector.tensor_copy(C(13), qf[:, :])  # f32 -> int32
                ts(C(12), C(13), NSUB - 1, A.bitwise_and)
                w_cv = ts(C(12), C(12), 10, A.logical_shift_left)

                li_cv, (cv,) = nc.values_load_multi_w_load_instructions(
                    C(12), engines=(DVE,), min_val=0, max_val=FREE - SUB,
                    skip_runtime_bounds_check=True,
                )
                for L in li_cv:
                    add_dep_helper(L.ins, w_cv.ins, sync=True, reason="cv load after write")

                # index of M within the winning 1024-wide window
                if "static_win" in _DBG:
                    win = power[:, 0:SUB]
                else:
                    win = power[:, bass.ds(cv, SUB)]
                wmax8 = spool.tile([128, 8], F32, tag="wmax8")
                nc.vector.max(out=wmax8[:, :], in_=win)
                idx8 = spool.tile([128, 8], U32, tag="idx8")
                nc.vector.max_index(out=idx8[:, :], in_max=wmax8[:, :], in_values=win)

                # flat = BIG2 - max_p(eq*BIG2 - (p*8192 + k_p)) + c1024
                idxf = spool.tile([128, 1], F32, tag="idxf")
                nc.vector.tensor_copy(idxf[:, :], idx8[:, 0:1])  # uint32 -> f32
                bi = spool.tile([128, 1], F32, tag="bi")
                nc.vector.tensor_tensor(out=bi[:, :], in0=idxf[:, :], in1=base[:, :], op=A.add)
                cand2 = spool.tile([128, 1], F32, tag="cand2")
                nc.vector.scalar_tensor_tensor(
                    out=cand2[:, :], in0=eq[:, :], scalar=BIG2, in1=bi[:, :],
                    op0=A.mult, op1=A.subtract,
                )
                allf = spool.tile([128, 1], F32, tag="allf")
                nc.gpsimd.partition_all_reduce(allf[:, :], cand2[:, :], 128, ReduceOp.max)
                f1 = spool.tile([1, 1], F32, tag="f1")
                ts(f1[:, :], allf[0:1, 0:1], BIG2, A.subtract, -1.0, A.mult)
                nc.vector.tensor_copy(C(14), f1[:, :])  # f32 -> int32
                nc.vector.tensor_tensor(out=C(0), in0=C(14), in1=C(12), op=A.add)

                # decompose flat -> window params
                ts(C(1), C(0), 14, A.logical_shift_right)             # d
                ts(C(2), C(0), 7, A.logical_shift_right, 127, A.bitwise_and)  # h
                ts(C(3), C(0), 127, A.bitwise_and)                    # w
                ts(C(4), C(2), 112, A.add)
                ts(C(4), C(4), 127, A.bitwise_and)                    # h0
                ts(C(5), C(3), 112, A.add)
                w_w0 = ts(C(5), C(5), 127, A.bitwise_and)             # w0
                w_sh = ts(C(6), C(4), 31, A.bitwise_and)              # sh
                ts(C(7), C(5), -1, A.mult, 128, A.add)                # 128 - w0
                w_a32 = ts(C(7), C(7), 32, A.min)                     # a32
                ts(C(9), C(1), 56, A.add)                             # d + 56
                ts(C(10), C(4), 5, A.logical_shift_right)             # c0
                ts(C(11), C(4), 31, A.add)
                ts(C(11), C(11), 127, A.bitwise_and)
                ts(C(11), C(11), 5, A.logical_shift_right)            # c1

                # gather row indices: 64 idxs in wrapped [16, 4] int16 layout;
                # descriptor n = chunk*32 + c*16 + i -> partition n
                bc3 = spool.tile([16, 3], I32, tag="bc3")
                nc.gpsimd.partition_broadcast(bc3[:, :], scal[0:1, 16 * s + 9 : 16 * s + 12], channels=16)
                dterm = spool.tile([16, 1], I32, tag="dterm")
                nc.vector.tensor_tensor(out=dterm[:, :], in0=iotaq[:, :], in1=bc3[:, 0:1], op=A.add)
                ts(dterm[:, :], dterm[:, :], 63, A.bitwise_and, 2, A.logical_shift_left)
                idx32 = spool.tile([16, 6], I32, tag="idx32")
                nc.vector.tensor_tensor(out=idx32[:, 4:5], in0=dterm[:, :], in1=bc3[:, 1:2], op=A.add)
                nc.vector.tensor_tensor(out=idx32[:, 5:6], in0=dterm[:, :], in1=bc3[:, 2:3], op=A.add)
                for t in range(4):
                    ts(idx32[:, t : t + 1], idx32[:, 4 + t // 2 : 5 + t // 2], (2 * s + t % 2) * 256, A.add)
                idx16 = spool.tile([16, 4], I16, tag="idx16")
                nc.vector.tensor_copy(idx16[:, :], idx32[:, 0:4])
                idxrep = spool.tile([128, 4], I16, tag="idxrep")
                for g in range(8):
                    nc.sync.dma_start(idxrep[16 * g : 16 * g + 16, :], idx16[:, :])

                # two 32-descriptor gathers land both h-chunks on the SAME
                # partition q = c*16 + d: chunk c0 rows at free 0:4096, chunk
                # c1 rows at free 4096:8192 -- the 64-row merged h-space is
                # contiguous per partition, no merge copies needed
                nc.gpsimd.dma_gather(
                    out_ap=G[:, 0:4096].rearrange("p (a b) -> p a b", a=1),
                    in_ap=xrows,
                    idxs_ap=idxrep[:, 0:2],
                    num_idxs=32,
                    num_idxs_reg=32,
                    elem_size=4096,
                )
                nc.gpsimd.dma_gather(
                    out_ap=G[:, 4096:8192].rearrange("p (a b) -> p a b", a=1),
                    in_ap=xrows,
                    idxs_ap=idxrep[:, 2:4],
                    num_idxs=32,
                    num_idxs_reg=32,
                    elem_size=4096,
                )

                # window extraction: 2 register-offset [32,32,32] copies
                li_v, (w0v, shv, a32v) = nc.values_load_multi_w_load_instructions(
                    scal[0:1, 16 * s + 5 : 16 * s + 8], engines=(DVE, ACT),
                    min_val=0, max_val=128, skip_runtime_bounds_check=True,
                )
                for L in li_v:
                    for W in (w_w0, w_sh, w_a32):
                        add_dep_helper(L.ins, W.ins, sync=True, reason="reg load after scal write")
                w0v = nc.s_assert_within(w0v, 0, 127, skip_runtime_assert=True)
                shv = nc.s_assert_within(shv, 0, 32, skip_runtime_assert=True)
                a32v = nc.s_assert_within(a32v, 0, 32, skip_runtime_assert=True)

                g3 = G[:, 0:FREE].rearrange("p (r w) -> p r w", w=128)  # [128, 64, 128]
                # overlapping view: row stride 128 but 160-wide cols, so the
                # w-window ds stays in per-axis bounds; reads past a row's end
                # land in the next row and are patched by the fixup copy
                g3o = bass.AP(g3.tensor, g3.offset, [list(d) for d in g3.ap[:-1]] + [[1, 160]])
                out_sb = opool.tile([32, 32 * 64], F32, tag="ob")
                o3 = out_sb[:, :].rearrange("p (r w) -> p r w", w=64)  # [32, 32, 64]
                if "static_copy" in _DBG:
                    nc.vector.tensor_copy(o3[:, :, 0:32], g3[0:32, 0:32, 0:32])
                    nc.scalar.copy(o3[:, :, 32:64], g3[0:32, 0:32, 0:32])
                else:
                    # main copy: h-window rows x w-window cols
                    nc.vector.tensor_copy(o3[:, :, 0:32], g3o[0:32, bass.ds(shv, 32), bass.ds(w0v, 32)])
                    # w-wrap fixup: rewrite cols >= a32 with the row's first
                    # cols (lands in the row padding when there is no wrap)
                    nc.scalar.copy(o3[:, :, bass.ds(a32v, 32)], g3[0:32, bass.ds(shv, 32), 0:32])

                nc.sync.dma_start(y[32 * s : 32 * s + 32, :], o3[:, :, 0:32])

    nc.compile()
    return nc


def get_nc():
    if "nc" not in _cache:
        _cache["nc"] = _build()
    return _cache["nc"]


def kernel(x: np.ndarray, **run_kwargs) -> np.ndarray:
    assert x.shape == (32, 2, 64, 128, 128) and x.dtype == np.float32
    nc = get_nc()
    in_maps = []
    for c in range(N_CORES):
        xc = x[c * S_PER_CORE : (c + 1) * S_PER_CORE]           # [4, 2, 64, 128, 128]
        xc = np.ascontiguousarray(xc).reshape(N_VOLS, 128, FREE)
        in_maps.append({"x": xc})
    res = run_bass_kernel_spmd(nc, in_maps, core_ids=list(range(N_CORES)), **run_kwargs)
    out = np.empty((32, 2, 16, 32, 32), dtype=np.float32)
    for c in range(N_CORES):
        yc = res.results[c]["y"].reshape(S_PER_CORE, 2, 16, 32, 32)
        out[c * S_PER_CORE : (c + 1) * S_PER_CORE] = yc
    if run_kwargs:
        return out, res
    return out


# revision 28
# speedup vs baseline: 1.1625x; 1.1625x over previous
"""CenterCut2 Trainium2 kernel (v2).

For each sample b: find argmax of power = sum_c x[b,c]^2 over the (D,H,W)
volume, then extract the 16x32x32 window centered on the peak with circular
wraparound (equivalent to reference's per-sample roll + center crop).

Sharding: pure data parallelism, 4 samples per core across 8 cores.

Per-core device program (samples s=0..3, volumes v=2s+c laid out [128, 8192]
with flat voxel index = p*8192 + f = dd*16384 + hh*128 + w):

  1. Stream both channels in [128, 2, 2048] chunks; square in place on ACT;
     a single fused DVE tensor_tensor_reduce per 1024-wide sub-chunk both
     adds the channel squares into the power map AND emits the sub-chunk max
     (mb[128, 8]) -- no separate MAX8 pass.
  2. Global argmax via small reductions only: partition_all_reduce(max) for
     the global max M, an is_equal + iota + BIG-constant trick over mb to
     pick the lexicographically-lowest (partition, sub-chunk) containing M,
     then max_index over just the winning 1024-wide window (register-offset
     AP). Tie-breaks match jnp.argmax (lowest flat index).
  3. One 64-descriptor dma_gather (16KB each) pulls the two 32-row h-chunks
     per (channel, d-slice); rows land at partition chunk*32 + c*16 + d.
  4. Window extraction via 4 register-offset [32,32,32] copies into a padded
     [32, 64, 64] output tile: A/B split the h-window across the two 32-row
     chunks, C/D patch the w-wraparound columns (writes beyond the window
     land in tile padding). One strided DMA writes the sample's output.
"""
import os
import sys

sys.path.insert(0, "/opt/trn_rl_repo")

import numpy as np

_DBG = set(filter(None, os.environ.get("K_DBG", "").split(",")))

import concourse.bass as bass
import concourse.bacc as bacc
import concourse.mybir as mybir
from concourse.tile import TileContext
from concourse.tile_rust import add_dep_helper
from concourse.bass_utils import run_bass_kernel_spmd
from concourse.bass_isa import ReduceOp

F32 = mybir.dt.float32
I32 = mybir.dt.int32
I16 = mybir.dt.int16
U32 = mybir.dt.uint32
A = mybir.AluOpType
DVE = mybir.EngineType.DVE
ACT = mybir.EngineType.Activation

N_CORES = 8
S_PER_CORE = 4          # samples per core
N_VOLS = 2 * S_PER_CORE # channel volumes per core
VOL = 64 * 128 * 128    # voxels per volume
FREE = VOL // 128       # 8192 free elements per partition
CHUNK = 2048            # streaming chunk per channel (1 MiB per DMA)
SUB = 1024              # ttr accum granularity (8 sub-chunks per sample)
NSUB = FREE // SUB
BIG2 = float(1 << 22)   # > max flat index (2^20)
BIG3 = float(1 << 21)   # > max q index (1031)

_cache = {}


def _build():
    nc = bacc.Bacc("TRN2", target_bir_lowering=False, debug=False, num_devices=N_CORES)
    x = nc.dram_tensor("x", [N_VOLS, 128, FREE], F32, kind="ExternalInput")
    y = nc.dram_tensor("y", [128, 1024], F32, kind="ExternalOutput")

    iota_base_c = nc.inline_tensor(
        (np.arange(128, dtype=np.float32) * FREE).reshape(128, 1), name="iota_base"
    )
    iota_q8_c = nc.inline_tensor(
        (np.arange(128, dtype=np.float32)[:, None] * NSUB
         + np.arange(NSUB, dtype=np.float32)[None, :]),
        name="iota_q8",
    )
    iotap16_c = nc.inline_tensor(
        (np.arange(128, dtype=np.int32) % 16).reshape(128, 1), name="iotap16"
    )
    # gather source view: [2048 rows, 4096] -- row = vol*256 + dd*4 + hchunk
    xrows = x.ap().rearrange("v p (a b) -> (v p a) b", a=2)

    with TileContext(nc) as tc:
        with (
            tc.tile_pool(name="xc", bufs=4) as xpool,
            tc.tile_pool(name="pw", bufs=2) as ppool,
            tc.tile_pool(name="sm", bufs=2) as spool,
            tc.tile_pool(name="ob", bufs=2) as opool,
            tc.tile_pool(name="big", bufs=1) as bpool,
        ):
            base = bpool.tile([128, 1], F32, tag="base")
            nc.sync.dma_start(base[:, :], iota_base_c.ap()[:, :])
            iq8 = bpool.tile([128, NSUB], F32, tag="iq8")
            nc.sync.dma_start(iq8[:, :], iota_q8_c.ap()[:, :])
            iotap16 = bpool.tile([128, 1], I32, tag="iotap16")
            nc.sync.dma_start(iotap16[:, :], iotap16_c.ap()[:, :])
            scal = bpool.tile([1, 64], I32, tag="scal")
            # persistent gather landing tile; zero once so the dynamic
            # over-reads in the extraction copies never touch uninit memory
            # (padded past 64*128 so the overlapping 160-wide view fits)
            G = bpool.tile([128, FREE + 256], F32, tag="gt")
            nc.vector.memset(G[:, :], 0.0)

            def ts(dst, src, s1, op0, s2=None, op1=None):
                kw = {}
                if s2 is not None:
                    kw = dict(scalar2=s2, op1=op1)
                else:
                    kw = dict(scalar2=None)
                return nc.vector.tensor_scalar(
                    out=dst, in0=src, scalar1=s1, op0=op0, **kw
                )

            for s in range(S_PER_CORE):
                power = ppool.tile([128, FREE], F32, tag="pw")
                m8 = spool.tile([128, NSUB * 8], F32, tag="m8")
                # [128, NSUB] view of the per-sub-chunk maxes (stride 8: first
                # element of each vector.max output group)
                _mbb = m8[:, 0:NSUB]
                mb = bass.AP(_mbb.tensor, _mbb.offset, [list(_mbb.ap[0])] + [[8, NSUB]])
                # stream both channels, square in place, fused add+max
                for k in range(FREE // CHUNK):
                    sl = slice(k * CHUNK, (k + 1) * CHUNK)
                    xc = xpool.tile([128, 2, CHUNK], F32, tag="xc")
                    nc.sync.dma_start(xc[:, 0, :], x[2 * s, :, sl])
                    nc.sync.dma_start(xc[:, 1, :], x[2 * s + 1, :, sl])
                    xflat = xc[:, :, :].rearrange("p c f -> p (c f)")
                    nc.scalar.square(xflat, xflat)  # in place
                    for j in range(CHUNK // SUB):
                        c = k * (CHUNK // SUB) + j
                        jj = slice(j * SUB, (j + 1) * SUB)
                        if "ttr" in _DBG:
                            nc.vector.tensor_tensor_reduce(
                                out=power[:, c * SUB : (c + 1) * SUB],
                                in0=xc[:, 0, jj],
                                in1=xc[:, 1, jj],
                                scale=1.0,
                                scalar=-1.0,
                                op0=A.add,
                                op1=A.max,
                                accum_out=m8[:, 8 * c : 8 * c + 1],
                            )
                        else:
                            nc.vector.tensor_tensor(
                                out=power[:, c * SUB : (c + 1) * SUB],
                                in0=xc[:, 0, jj], in1=xc[:, 1, jj], op=A.add,
                            )
                            nc.vector.max(
                                out=m8[:, 8 * c : 8 * c + 8],
                                in_=power[:, c * SUB : (c + 1) * SUB],
                            )

                # ---- global argmax via small reductions ----
                pm8 = spool.tile([128, 8], F32, tag="pm8")
                nc.vector.max(out=pm8[:, :], in_=mb[:, :])
                m = pm8[:, 0:1]
                M = spool.tile([128, 1], F32, tag="M")
                nc.gpsimd.partition_all_reduce(M[:, :], m, 128, ReduceOp.max)
                eq = spool.tile([128, 1], F32, tag="eq")
                nc.vector.tensor_tensor(out=eq[:, :], in0=m, in1=M[:, :], op=A.is_equal)

                # winning (partition, sub-chunk), lexicographic min
                eqc = spool.tile([128, NSUB], F32, tag="eqc")
                mb_b, M_b = bass.broadcast_tensor_aps(mb[:, :], M[:, :])
                nc.vector.tensor_tensor(out=eqc[:, :], in0=mb_b, in1=M_b, op=A.is_equal)
                candq = spool.tile([128, NSUB], F32, tag="candq")
                nc.vector.scalar_tensor_tensor(
                    out=candq[:, :], in0=eqc[:, :], scalar=BIG3, in1=iq8[:, :],
                    op0=A.mult, op1=A.subtract,
                )
                cq8 = spool.tile([128, 8], F32, tag="cq8")
                nc.vector.max(out=cq8[:, :], in_=candq[:, :])
                allq = spool.tile([128, 1], F32, tag="allq")
                nc.gpsimd.partition_all_reduce(allq[:, :], cq8[:, 0:1], 128, ReduceOp.max)

                def C(j):
                    return scal[:, 16 * s + j : 16 * s + j + 1]

                # q = BIG3 - allq; c1024 = (q & 7) << 10
                qf = spool.tile([1, 1], F32, tag="qf")
                ts(qf[:, :], allq[0:1, 0:1], BIG3, A.subtract, -1.0, A.mult)
                nc.vector.tensor_copy(C(13), qf[:, :])  # f32 -> int32
                ts(C(12), C(13), NSUB - 1, A.bitwise_and)
                w_cv = ts(C(12), C(12), 10, A.logical_shift_left)

                li_cv, (cv,) = nc.values_load_multi_w_load_instructions(
                    C(12), engines=(DVE,), min_val=0, max_val=FREE - SUB,
                    skip_runtime_bounds_check=True,
                )
                for L in li_cv:
                    add_dep_helper(L.ins, w_cv.ins, sync=True, reason="cv load after write")

                # index of M within the winning 1024-wide window
                if "static_win" in _DBG:
                    win = power[:, 0:SUB]
                else:
                    win = power[:, bass.ds(cv, SUB)]
                wmax8 = spool.tile([128, 8], F32, tag="wmax8")
                nc.vector.max(out=wmax8[:, :], in_=win)
                idx8 = spool.tile([128, 8], U32, tag="idx8")
                nc.vector.max_index(out=idx8[:, :], in_max=wmax8[:, :], in_values=win)

                # flat = BIG2 - max_p(eq*BIG2 - (p*8192 + k_p)) + c1024
                idxf = spool.tile([128, 1], F32, tag="idxf")
                nc.vector.tensor_copy(idxf[:, :], idx8[:, 0:1])  # uint32 -> f32
                bi = spool.tile([128, 1], F32, tag="bi")
                nc.vector.tensor_tensor(out=bi[:, :], in0=idxf[:, :], in1=base[:, :], op=A.add)
                cand2 = spool.tile([128, 1], F32, tag="cand2")
                nc.vector.scalar_tensor_tensor(
                    out=cand2[:, :], in0=eq[:, :], scalar=BIG2, in1=bi[:, :],
                    op0=A.mult, op1=A.subtract,
                )
                allf = spool.tile([128, 1], F32, tag="allf")
                nc.gpsimd.partition_all_reduce(allf[:, :], cand2[:, :], 128, ReduceOp.max)
                f1 = spool.tile([1, 1], F32, tag="f1")
                ts(f1[:, :], allf[0:1, 0:1], BIG2, A.subtract, -1.0, A.mult)
                nc.vector.tensor_copy(C(14), f1[:, :])  # f32 -> int32
                nc.vector.tensor_tensor(out=C(0), in0=C(14), in1=C(12), op=A.add)

                # decompose flat -> gather params first (so the gathers can
                # dispatch before the copy-offset math runs)
                ts(C(1), C(0), 14, A.logical_shift_right)             # d
                ts(C(2), C(0), 7, A.logical_shift_right, 127, A.bitwise_and)  # h
                ts(C(4), C(2), 112, A.add)
                ts(C(4), C(4), 127, A.bitwise_and)                    # h0
                ts(C(9), C(1), 56, A.add)                             # d + 56
                ts(C(10), C(4), 5, A.logical_shift_right)             # c0
                ts(C(11), C(4), 31, A.add)
                ts(C(11), C(11), 127, A.bitwise_and)
                ts(C(11), C(11), 5, A.logical_shift_right)            # c1

                # gather row indices, built directly in the wrapped+replicated
                # [128, 4] int16 layout: value at (r, j) = idx of descriptor
                # n = j*16 + r%16, i.e. (d0 + r%16 windowed d-slice, chunk
                # c0/c1 for j<2/j>=2, channel j%2)
                bc3 = spool.tile([128, 3], I32, tag="bc3")
                nc.gpsimd.partition_broadcast(bc3[:, :], scal[0:1, 16 * s + 9 : 16 * s + 12], channels=128)
                dt128 = spool.tile([128, 1], I32, tag="dt128")
                nc.vector.tensor_tensor(out=dt128[:, :], in0=iotap16[:, :], in1=bc3[:, 0:1], op=A.add)
                ts(dt128[:, :], dt128[:, :], 63, A.bitwise_and, 2, A.logical_shift_left)
                idx4 = spool.tile([128, 4], I32, tag="idx4")
                for t in range(4):
                    nc.vector.scalar_tensor_tensor(
                        out=idx4[:, t : t + 1], in0=bc3[:, 1 + t // 2 : 2 + t // 2],
                        scalar=(2 * s + t % 2) * 256, in1=dt128[:, :],
                        op0=A.add, op1=A.add,
                    )
                idx16 = spool.tile([128, 4], I16, tag="idx16")
                nc.vector.tensor_copy(idx16[:, :], idx4[:, :])

                # two 32-descriptor gathers land both h-chunks on the SAME
                # partition q = c*16 + d: chunk c0 rows at free 0:4096, chunk
                # c1 rows at free 4096:8192 -- the 64-row merged h-space is
                # contiguous per partition, no merge copies needed
                nc.gpsimd.dma_gather(
                    out_ap=G[:, 0:4096].rearrange("p (a b) -> p a b", a=1),
                    in_ap=xrows,
                    idxs_ap=idx16[:, 0:2],
                    num_idxs=32,
                    num_idxs_reg=32,
                    elem_size=4096,
                )
                nc.gpsimd.dma_gather(
                    out_ap=G[:, 4096:8192].rearrange("p (a b) -> p a b", a=1),
                    in_ap=xrows,
                    idxs_ap=idx16[:, 2:4],
                    num_idxs=32,
                    num_idxs_reg=32,
                    elem_size=4096,
                )

                # copy-offset params (overlap the gather DMAs)
                ts(C(3), C(0), 127, A.bitwise_and)                    # w
                ts(C(5), C(3), 112, A.add)
                w_w0 = ts(C(5), C(5), 127, A.bitwise_and)             # w0
                w_sh = ts(C(6), C(4), 31, A.bitwise_and)              # sh
                ts(C(7), C(5), -1, A.mult, 128, A.add)                # 128 - w0
                w_a32 = ts(C(7), C(7), 32, A.min)                     # a32

                # window extraction: 2 register-offset [32,32,32] copies
                li_v, (w0v, shv, a32v) = nc.values_load_multi_w_load_instructions(
                    scal[0:1, 16 * s + 5 : 16 * s + 8], engines=(DVE, ACT),
                    min_val=0, max_val=128, skip_runtime_bounds_check=True,
                )
                for L in li_v:
                    for W in (w_w0, w_sh, w_a32):
                        add_dep_helper(L.ins, W.ins, sync=True, reason="reg load after scal write")
                w0v = nc.s_assert_within(w0v, 0, 127, skip_runtime_assert=True)
                shv = nc.s_assert_within(shv, 0, 32, skip_runtime_assert=True)
                a32v = nc.s_assert_within(a32v, 0, 32, skip_runtime_assert=True)

                g3 = G[:, 0:FREE].rearrange("p (r w) -> p r w", w=128)  # [128, 64, 128]
                # overlapping view: row stride 128 but 160-wide cols, so the
                # w-window ds stays in per-axis bounds; reads past a row's end
                # land in the next row and are patched by the fixup copy
                g3o = bass.AP(g3.tensor, g3.offset, [list(d) for d in g3.ap[:-1]] + [[1, 160]])
                out_sb = opool.tile([32, 32 * 64], F32, tag="ob")
                o3 = out_sb[:, :].rearrange("p (r w) -> p r w", w=64)  # [32, 32, 64]
                if "static_copy" in _DBG:
                    nc.vector.tensor_copy(o3[:, :, 0:32], g3[0:32, 0:32, 0:32])
                    nc.scalar.copy(o3[:, :, 32:64], g3[0:32, 0:32, 0:32])
                else:
                    # main copy: h-window rows x w-window cols
                    nc.vector.tensor_copy(o3[:, :, 0:32], g3o[0:32, bass.ds(shv, 32), bass.ds(w0v, 32)])
                    # w-wrap fixup: rewrite cols >= a32 with the row's first
                    # cols (lands in the row padding when there is no wrap)
                    nc.scalar.copy(o3[:, :, bass.ds(a32v, 32)], g3[0:32, bass.ds(shv, 32), 0:32])

                # dispatch on the ACT HWDGE queue so the output write never
                # queues the stream DMAs on Sync behind the extraction deps
                nc.scalar.dma_start(y[32 * s : 32 * s + 32, :], o3[:, :, 0:32])

    nc.compile()
    return nc


def get_nc():
    if "nc" not in _cache:
        _cache["nc"] = _build()
    return _cache["nc"]


def kernel(x: np.ndarray, **run_kwargs) -> np.ndarray:
    assert x.shape == (32, 2, 64, 128, 128) and x.dtype == np.float32
    nc = get_nc()
    in_maps = []
    for c in range(N_CORES):
        xc = x[c * S_PER_CORE : (c + 1) * S_PER_CORE]           # [4, 2, 64, 128, 128]
        xc = np.ascontiguousarray(xc).reshape(N_VOLS, 128, FREE)
        in_maps.append({"x": xc})
    res = run_bass_kernel_spmd(nc, in_maps, core_ids=list(range(N_CORES)), **run_kwargs)
    out = np.empty((32, 2, 16, 32, 32), dtype=np.float32)
    for c in range(N_CORES):
        yc = res.results[c]["y"].reshape(S_PER_CORE, 2, 16, 32, 32)
        out[c * S_PER_CORE : (c + 1) * S_PER_CORE] = yc
    if run_kwargs:
        return out, res
    return out
